# revision 66
# baseline (speedup 1.0000x reference)
"""BetaMoE Trainium2 Bass kernel, v2.

Self-contained: hardcodes B=256,T=60,C=512,E=10,K=5,H=8, shards batch over
8 NeuronCores (32 rows each), pads T->64 so 2 batch rows = 1 partition tile.

Structure (per core, BL=32 rows, 2048 padded tokens):
- Attention/router path stays fp32r (router top-5 margins are ~1e-4, any
  quantization flips expert selection):
  scores  = qkT.T @ dataT, batched 4 token-tiles per matmul (N=512);
  softmax via an HBM roundtrip into a [16, 2, H, T] token-tile layout;
  ctx via attention-pooled data: ad = attnones.T @ datab (token-partition
  data), adT by PE transpose, ctx = wvT.T @ adT per head -> ctxT directly;
  router/beta heads from ctxT with the ones-row bias trick.
- Expert mm1 in fp8e4 DoubleRow with 3-term residual splitting
  (d_hi*w_hi + d_lo*w_hi + d_hi*w_lo), each operand = fp8(x) + fp8 residual,
  which lands bf16-level accuracy at 4x PE rate (0.5 cyc/row, 2 k-tiles per
  instruction).
- relu evictions of h to bf16 swh, W = router_prob * beta time weight is
  applied via the pooling matmul's moving operand (Wones, per-expert
  weighted block-ones), pooled gT accumulates transposed [ek, b] so mm2
  needs no extra transposes.
- LayerNorm rstd via exp(-0.5*ln(var)) so every activation function used
  (exp/ln/abs/relu/square) lives in one ACT table set.
"""

import math

import numpy as np
import ml_dtypes

import concourse.bass as bass
import concourse.bacc as bacc
import concourse.mybir as mybir
import concourse.tile as tile
from concourse.bass_utils import run_bass_kernel_spmd

F32 = mybir.dt.float32
F32R = mybir.dt.float32r
BF16 = mybir.dt.bfloat16
FP8 = mybir.dt.float8e4
AF = mybir.ActivationFunctionType
ALU = mybir.AluOpType
AX = mybir.AxisListType
DR = mybir.MatmulPerfMode.DoubleRow

B, T, C, E, TOPK, H = 256, 60, 512, 10, 5, 8
DH = C // H          # 64
TP = 64              # padded T
NCORE = 8
BL = B // NCORE      # 32
NTOK = BL * TP       # 2048
NT = NTOK // 128     # 16 token tiles
CH = C // 2          # 256
EC = E * CH          # 2560
NKC = C // 128       # 4 k-tiles over C
NCH = 5              # mm1 column chunks of 512
LN2PI_HALF = 0.5 * math.log(2.0 * math.pi)

# cpack layout (per-partition fp32 words)
C_EYE = 0                  # (128, 128) identity fp32
C_LT = C_EYE + 128         # (32, 64) log(t+1e-12)
C_L1 = C_LT + TP           # (32, 64) log(1-t+1e-12)
C_OB = C_L1 + TP           # (128, 62) onesBig sliding block-ones
C_OC = C_OB + 62           # (128, 32) pad-masked ones columns
C_OR = C_OC + 32           # (1, 128) ones row (partition 0)
C_O2 = C_OR + 128          # (128, 2) pad-masked half-selector ones
C_O12 = C_O2 + 2           # (1, 32) ones row for the head bias matmul
C_IM = C_O12 + 32          # (128, 16) attnones i-block/pad mask
C_M16 = C_IM + 16          # (128, 1) constant -16 softmax bound bias
C_ONE = C_M16 + 1          # (128, 2) plain ones columns
CPK = C_ONE + 2

# apack layout
A_WV = 0                   # (128, 4, 512) wv.T k-tiles
A_WH = A_WV + NKC * C      # (64, 8, 30) router+beta heads, per-head rows
A_HB = A_WH + 8 * 30       # (1, 30) head bias row
APK = A_HB + 30

_CACHE = {}
DEBUG = False


def _build_program(use_b2, use_ln):
    nc = bacc.Bacc("TRN2", target_bir_lowering=False, debug=False,
                   enable_asserts=False, num_devices=NCORE)

    def inp(name, shape, dt=F32):
        return nc.dram_tensor(name, list(shape), dt, kind="ExternalInput")

    d = {}
    d["d_qkT"] = inp("qkT", (128, NKC * BL * H), F32R)
    d["d_cpack"] = inp("cpack", (128, CPK), F32R)
    d["d_dT8h"] = inp("dT8h", (128, NKC * NTOK), FP8)
    d["d_dT8l"] = inp("dT8l", (128, NKC * NTOK), FP8)
    d["d_w18h"] = inp("w18h", (128, NKC * EC), FP8)
    d["d_w18l"] = inp("w18l", (128, NKC * EC), FP8)
    d["d_dataT"] = inp("dataT", (128, NKC * NTOK), F32R)
    d["d_apack"] = inp("apack", (128, APK), F32R)
    d["d_datab"] = inp("datab", (128, NT * C), F32R)
    d["d_w2"] = inp("w2catT", (128, 20 * C), BF16)
    if use_b2:
        d["d_b2"] = inp("b2cat", (E, C), F32R)
    if use_ln:
        d["d_lng"] = inp("lng", (BL, 2 * C))

    d["d_out"] = nc.dram_tensor("out", [BL, C], F32, kind="ExternalOutput")
    if DEBUG:
        d["d_dbg"] = nc.dram_tensor("dbg", [128, 1024], F32,
                                    kind="ExternalOutput")
    # HBM scratch for the W -> wp partition shuffle
    d["w_scr"] = nc.dram_tensor("w_scr", [BL, TP * E], F32, kind="Internal")

    with tile.TileContext(nc) as tc:
        _emit(tc, d, use_b2, use_ln)
    nc.compile()
    return nc


def _emit(tc, d, use_b2, use_ln):
    nc = tc.nc
    dma = nc.sync.dma_start          # input streaming queue
    dmaA = nc.scalar.dma_start       # attention/chain roundtrip queue

    with tc.tile_pool(name="const", bufs=1) as cp, \
         tc.tile_pool(name="small", bufs=1) as sp, \
         tc.tile_pool(name="mm1ps", bufs=1, space="PSUM") as hpp:
        # ---- persistent tiles + input DMAs in bus order ----
        qkT_f = cp.tile([128, NKC * BL * H], F32R, tag="qkT")
        qkT = qkT_f[:].rearrange("p (k n) -> p k n", k=NKC)
        cpk = cp.tile([128, CPK], F32R, tag="cpk")

        w18h_f = cp.tile([128, NKC * EC], FP8, tag="w18h")
        w18l_f = cp.tile([128, NKC * EC], FP8, tag="w18l")
        dT8h_f = cp.tile([128, NKC * NTOK], FP8, tag="dT8h")
        dT8l_f = cp.tile([128, NKC * NTOK], FP8, tag="dT8l")
        w18h = w18h_f[:].rearrange("p (k n) -> p k n", k=NKC)
        w18l = w18l_f[:].rearrange("p (k n) -> p k n", k=NKC)
        dT8h = dT8h_f[:].rearrange("p (k n) -> p k n", k=NKC)
        dT8l = dT8l_f[:].rearrange("p (k n) -> p k n", k=NKC)
        w18h_d = d["d_w18h"].ap().rearrange("p (k n) -> p k n", k=NKC)
        w18l_d = d["d_w18l"].ap().rearrange("p (k n) -> p k n", k=NKC)
        dT8h_d = d["d_dT8h"].ap().rearrange("p (k n) -> p k n", k=NKC)
        dT8l_d = d["d_dT8l"].ap().rearrange("p (k n) -> p k n", k=NKC)
        # first mm1 chunk's weights + first data tokens, then stream the rest
        dma(w18h[:, :, 0:512], w18h_d[:, :, 0:512])
        dma(w18l[:, :, 0:512], w18l_d[:, :, 0:512])
        for ck in range(2):
            tsl = slice(ck * 512, (ck + 1) * 512)
            dma(dT8h[:, :, tsl], dT8h_d[:, :, tsl])
            dma(dT8l[:, :, tsl], dT8l_d[:, :, tsl])
        dma(qkT_f[:], d["d_qkT"].ap())
        dma(cpk[:], d["d_cpack"].ap())
        for ck in range(2, 4):
            tsl = slice(ck * 512, (ck + 1) * 512)
            dma(dT8h[:, :, tsl], dT8h_d[:, :, tsl])
            dma(dT8l[:, :, tsl], dT8l_d[:, :, tsl])

        eyef = cpk[:, C_EYE:C_EYE + 128]
        logt = cpk[0:BL, C_LT:C_LT + TP].bitcast(F32)
        log1mt = cpk[0:BL, C_L1:C_L1 + TP].bitcast(F32)
        onesBig = cpk[:, C_OB:C_OB + 62].bitcast(F32)
        onescol = cpk[:, C_OC:C_OC + 32]
        onesrow = cpk[0:1, C_OR:C_OR + 128]
        ones2 = cpk[:, C_O2:C_O2 + 2].bitcast(F32)
        ones12 = cpk[0:1, C_O12:C_O12 + 32]
        imask = cpk[:, C_IM:C_IM + 16]
        m16 = cpk[:, C_M16:C_M16 + 1].bitcast(F32)
        onep = cpk[:, C_ONE:C_ONE + 2]

        # ---- persistent working tiles ----
        swh = cp.tile([128, NT * EC], BF16, tag="swh", name="swh")
        swhv = swh[:].rearrange("p (j n) -> p j n", j=NT)
        attno = sp.tile([128, NT, BL], F32R, tag="attno", name="attno")
        adT = sp.tile([128, NKC, 2 * 128], F32R, tag="adT")
        ctxT2 = sp.tile([64, H, BL], F32R, tag="ctxT2")
        heads = sp.tile([BL, 30], F32, tag="heads")
        probs = sp.tile([BL, E], F32, tag="probs")
        p_sel = sp.tile([BL, E], F32, tag="p_sel")
        W = sp.tile([BL, E, TP], F32, tag="W")
        wp = sp.tile([128, NT, E], F32, tag="wp")
        wones = sp.tile([128, NT, E * 2], BF16, tag="wones", name="wones")
        gT = sp.tile([128, 20, BL], BF16, tag="gT")
        out_sb = sp.tile([BL, C], F32, tag="out_sb", name="out_sb")

        def mm1_nch(nch, eng_cycle):
            csl = slice(nch * 512, (nch + 1) * 512)
            for jt in range(NT):
                ps = hpp.tile([128, 512], F32, tag="hp", bufs=4, name="ps")
                tsl = slice(jt * 128, (jt + 1) * 128)
                k = 0
                for dt_, wt_ in ((dT8h, w18h), (dT8l, w18h), (dT8h, w18l)):
                    for pp in range(2):
                        ksl = slice(2 * pp, 2 * pp + 2)
                        nc.tensor.matmul(
                            ps[:], dt_[:, ksl, tsl], wt_[:, ksl, csl],
                            start=(k == 0), stop=(k == 5), perf_mode=DR)
                        k += 1
                eng = eng_cycle[jt % len(eng_cycle)]
                if eng == "act":
                    nc.scalar.activation(swhv[:, jt, csl], ps[:], AF.Relu)
                else:
                    nc.vector.tensor_relu(swhv[:, jt, csl], ps[:])

        # ================= mm1 nch0 (fills PE while attention data loads)
        mm1_nch(0, ("act", "vector"))

        def w18_chunk(nch):
            csl = slice(nch * 512, (nch + 1) * 512)
            dma(w18h[:, :, csl], w18h_d[:, :, csl])
            dma(w18l[:, :, csl], w18l_d[:, :, csl])

        # ================= scores with fused softmax (no HBM roundtrip)
        # exp(s - M) is applied during PSUM eviction with M = row-max over
        # the whole 4-tile group row (a valid upper bound, softmax-invariant);
        # attnones gets the unnormalized exps via PE transposes and the
        # 1/rsum normalization is folded into the adT eviction.
        attnov = attno[:]
        nc.gpsimd.memset(attno[:].bitcast(F32), 0.0)
        with tc.tile_pool(name="pdataT", bufs=1) as pdT:
            dataT_d = d["d_dataT"].ap().rearrange("p (k n) -> p k n", k=NKC)
            apk = cp.tile([128, APK], F32R, tag="apk")
            wvT = apk[:, A_WV:A_WV + NKC * C].rearrange(
                "p (k n) -> p k n", k=NKC)
            wheads2 = apk[0:64, A_WH:A_WH + 8 * 30].rearrange(
                "p (k n) -> p k n", k=8)
            hbias = apk[0:1, A_HB:A_HB + 30]

            with tc.tile_pool(name="sps", bufs=1, space="PSUM") as spp:
                for g in range(4):
                    dataTg = pdT.tile([128, NKC, 512], F32R, tag="dataTg",
                                      bufs=4, name="dataTg")
                    dma(dataTg[:],
                        dataT_d[:, :, g * 512:(g + 1) * 512])
                    ps_s = spp.tile([64, 512], F32, tag="ps_s", bufs=2,
                                    name="ps_s")
                    for kt in range(NKC):
                        nc.tensor.matmul(
                            ps_s[:],
                            qkT[:, kt, g * 64:(g + 1) * 64],
                            dataTg[:, kt, :],
                            start=(kt == 0), stop=(kt == NKC - 1))
                    # fixed M=16 >= any row max (|scores| <~ 10): softmax
                    # ratios are M-invariant and rsum uses the same M, so no
                    # row-max reduction is needed (saves a serial DVE stage)
                    essb = sp.tile([64, 512], F32R, tag="essb", bufs=2,
                                   name="essb")
                    nc.scalar.activation(essb[:], ps_s[:], AF.Exp,
                                         bias=m16[0:64, :])
                    for r in range(4):
                        jt = 4 * g + r
                        tp16 = spp.tile([128, 64], F32R, tag="saux",
                                        bufs=2, name="tp16")
                        nc.tensor.transpose(
                            tp16[:],
                            essb[:, 128 * r:128 * r + 128],
                            eyef[0:64, 0:64])
                        lb0 = 2 * (jt % 2)
                        blk = attno[:, jt, lb0 * 8:lb0 * 8 + 16]
                        nc.vector.tensor_mul(
                            blk, tp16[:, 16 * r:16 * r + 16], imask)

                # rsum per (b,h) as a row vector, then 1/rsum broadcast to
                # all 128 partitions via a K=1 PE matmul
                rs_ps = spp.tile([32, BL * H], F32, tag="saux",
                                 bufs=2, name="rs_ps")
                for jt in range(NT):
                    lb0 = 2 * (jt % 2)
                    nc.tensor.matmul(
                        rs_ps[:, 16 * jt:16 * jt + 16], onescol,
                        attnov[:, jt, lb0 * 8:lb0 * 8 + 16],
                        start=True, stop=True, skip_group_check=True)
                rrf = sp.tile([1, BL * H], F32, tag="rrf")
                nc.vector.reciprocal(rrf[:], rs_ps[0:1, :])
                rinvrow = sp.tile([1, BL * H], F32R, tag="rinvrow")
                nc.scalar.copy(rinvrow[:], rrf[:])
                rb_ps = spp.tile([128, BL * H], F32, tag="saux",
                                 bufs=2, name="rb_ps")
                nc.tensor.matmul(rb_ps[:], onesrow, rinvrow[:],
                                 start=True, stop=True)
                rinvB = sp.tile([128, BL * H], F32R, tag="rinvB")
                nc.vector.tensor_copy(rinvB[:], rb_ps[:].bitcast(F32R))

        w18_chunk(1)
        dma(apk[:], d["d_apack"].ap())

        # ---- datab loads (token-partition data for ad) + rest of w1 ----
        pdb_mgr = tc.tile_pool(name="pdatab", bufs=1)
        pdb = pdb_mgr.__enter__()
        datab_d = d["d_datab"].ap().rearrange("p (j n) -> p j n", j=NT)
        databcs = []
        for ck in range(4):
            databc = pdb.tile([128, 4, C], F32R, tag="databc",
                              bufs=4, name="databc")
            dma(databc[:], datab_d[:, 4 * ck:4 * ck + 4, :])
            databcs.append(databc)
        w18_chunk(2)
        w18_chunk(3)
        w18_chunk(4)

        # ================= mm1 nch1
        mm1_nch(1, ("act", "vector"))

        # ---- ad in transposed form: adT[c, bh] += datab.T @ attnones
        # per token tile (contracts tokens; base-0 PSUM, no transposes) ----
        with tc.tile_pool(name="adps", bufs=1, space="PSUM") as adp:
            adTps = adp.tile([128, NKC * BL * H], F32, tag="adTps",
                             name="adTps")
            adTv = adTps[:].rearrange("p (k n) -> p k n", k=NKC)
            for ck in range(4):
                databc = databcs[ck]
                for jl in range(4):
                    jt = 4 * ck + jl
                    base = 16 * jt
                    lb0 = 2 * (jt % 2)
                    for ct in range(NKC):
                        nc.tensor.matmul(
                            adTv[:, ct, base:base + 16],
                            databc[:, jl, ct * 128:(ct + 1) * 128],
                            attnov[:, jt, lb0 * 8:lb0 * 8 + 16],
                            start=True, stop=True, skip_group_check=True)
            pdb_mgr.__exit__(None, None, None)
            # evict with the 1/rsum attention normalization fused in
            for ct in range(NKC):
                nc.vector.tensor_mul(adT[:, ct, :],
                                     adTv[:, ct, :].bitcast(F32R),
                                     rinvB[:])

            # ctx per head -> [64, h, b] column slices (base partition 0)
            ctxps = adp.tile([64, H * BL], F32, tag="ctxps", name="ctxps")
            cpv = ctxps[:].rearrange("p (h b) -> p h b", h=H)
            adTb = adT[:].rearrange("p k (b h8) -> p k b h8", h8=8)
            for h in range(H):
                for kt in range(NKC):
                    nc.tensor.matmul(
                        cpv[:, h, :], wvT[:, kt, h * 64:(h + 1) * 64],
                        adTb[:, kt, :, h],
                        start=(kt == 0), stop=(kt == NKC - 1))
            nc.scalar.copy(ctxT2[:], ctxps[:].bitcast(F32R))
            # heads: contract 64 rows per head + K=1 bias row
            psh = adp.tile([BL, 30], F32, tag="headps", name="psh")
            for h in range(H):
                nc.tensor.matmul(psh[:], ctxT2[:, h, :], wheads2[:, h, :],
                                 start=(h == 0), stop=False)
            nc.tensor.matmul(psh[:], ones12, hbias,
                             start=False, stop=True)
            nc.scalar.copy(heads[:], psh[:])

        if DEBUG:
            dbg = sp.tile([128, 1024], F32, tag="dbg", name="dbg")
            nc.gpsimd.memset(dbg[:], 0.0)
            nc.vector.tensor_copy(dbg[0:32, 0:30], heads[:])
            nc.vector.tensor_copy(dbg[:, 32:64], attno[:, 0, :].bitcast(F32))
            nc.vector.tensor_copy(dbg[:, 64:320],
                                  adT[:, 0, :].bitcast(F32))
            nc.vector.tensor_copy(dbg[:, 320:576], rinvB[:].bitcast(F32))
            nc.vector.tensor_copy(dbg[0:64, 576:608],
                                  ctxT2[:, 0, :].bitcast(F32))
            dmaA(d["d_dbg"].ap(), dbg[:])

        # ---- router probs, top-k, beta weights (baseline chain) ----
        hp2_mgr = tc.high_priority()
        hp2_mgr.__enter__()
        logits = heads[:, 0:E]
        rmax2 = sp.tile([BL, 1], F32, tag="rmax2")
        nc.vector.tensor_reduce(rmax2[:], logits, AX.X, ALU.max)
        nc.vector.tensor_scalar(probs[:], logits, rmax2[:], None,
                                ALU.subtract)
        nc.scalar.activation(probs[:], probs[:], AF.Exp)
        rsum2 = sp.tile([BL, 1], F32, tag="rsum2")
        nc.vector.tensor_reduce(rsum2[:], probs[:], AX.X, ALU.add)
        rinv2 = sp.tile([BL, 1], F32, tag="rinv2")
        nc.vector.reciprocal(rinv2[:], rsum2[:])
        nc.vector.tensor_scalar(probs[:], probs[:], rinv2[:], None,
                                ALU.mult)
        m8 = sp.tile([BL, 8], F32, tag="m8")
        nc.vector.max(m8[:], probs[:])
        nc.vector.tensor_scalar(p_sel[:], probs[:], m8[:, TOPK - 1:TOPK],
                                None, ALU.is_ge)
        nc.vector.tensor_mul(p_sel[:], p_sel[:], probs[:])
        msum = sp.tile([BL, 1], F32, tag="msum")
        nc.vector.tensor_reduce(msum[:], p_sel[:], AX.X, ALU.add)
        nc.vector.tensor_scalar_add(msum[:], msum[:], 1e-8)
        minv = sp.tile([BL, 1], F32, tag="minv")
        nc.vector.reciprocal(minv[:], msum[:])
        nc.vector.tensor_scalar(p_sel[:], p_sel[:], minv[:], None,
                                ALU.mult)

        x3 = sp.tile([BL, 30], F32, tag="x3")
        sp20 = sp.tile([BL, 2 * E], F32, tag="sp20")
        relu20 = sp.tile([BL, 2 * E], F32, tag="relu20")
        nc.scalar.activation(sp20[:], heads[:, E:30], AF.Abs)
        nc.scalar.activation(sp20[:], sp20[:], AF.Exp, scale=-1.0)
        nc.vector.tensor_scalar_add(sp20[:], sp20[:], 1.0)
        nc.scalar.activation(sp20[:], sp20[:], AF.Ln)
        nc.vector.tensor_scalar_max(relu20[:], heads[:, E:30], 0.0)
        nc.vector.tensor_add(sp20[:], sp20[:], relu20[:])
        sp2 = sp20[:].rearrange("p (e two) -> p e two", two=2)
        nc.vector.tensor_scalar_add(x3[:, 0:E],
                                    sp2[:, :, 0:1].squeeze(-1), 1e-6)
        nc.vector.tensor_scalar_add(x3[:, E:2 * E],
                                    sp2[:, :, 1:2].squeeze(-1), 1e-6)
        nc.vector.tensor_add(x3[:, 2 * E:30], x3[:, 0:E], x3[:, E:2 * E])
        lg = sp.tile([BL, 30], F32, tag="lg")
        pprod = sp.tile([BL, 30], F32, tag="pprod")
        ptmp = sp.tile([BL, 30], F32, tag="ptmp")
        ptmp2 = sp.tile([BL, 30], F32, tag="ptmp2")
        nc.vector.scalar_tensor_tensor(pprod[:], x3[:], 1.0, x3[:],
                                       op0=ALU.add, op1=ALU.mult)
        for base in (2, 4, 6):
            nc.vector.tensor_scalar_add(ptmp[:], x3[:], float(base + 1))
            nc.vector.scalar_tensor_tensor(ptmp2[:], x3[:], float(base),
                                           ptmp[:], op0=ALU.add,
                                           op1=ALU.mult)
            nc.vector.tensor_mul(pprod[:], pprod[:], ptmp2[:])
        z = sp.tile([BL, 30], F32, tag="z")
        nc.vector.tensor_scalar_add(z[:], x3[:], 8.0)
        lnz = sp.tile([BL, 30], F32, tag="lnz")
        nc.scalar.activation(lnz[:], z[:], AF.Ln)
        nc.scalar.activation(pprod[:], pprod[:], AF.Ln)
        r1 = sp.tile([BL, 30], F32, tag="r1")
        nc.vector.reciprocal(r1[:], z[:])
        r2 = sp.tile([BL, 30], F32, tag="r2")
        nc.vector.tensor_mul(r2[:], r1[:], r1[:])
        poly = sp.tile([BL, 30], F32, tag="poly")
        nc.vector.tensor_scalar(poly[:], r2[:], 1.0 / 1260.0, -1.0 / 360.0,
                                ALU.mult, ALU.add)
        nc.vector.tensor_mul(poly[:], poly[:], r2[:])
        nc.vector.tensor_scalar_add(poly[:], poly[:], 1.0 / 12.0)
        nc.vector.tensor_mul(poly[:], poly[:], r1[:])
        nc.vector.scalar_tensor_tensor(lg[:], z[:], -0.5, lnz[:],
                                       op0=ALU.add, op1=ALU.mult)
        nc.vector.tensor_sub(lg[:], lg[:], z[:])
        nc.vector.scalar_tensor_tensor(lg[:], lg[:], LN2PI_HALF, poly[:],
                                       op0=ALU.add, op1=ALU.add)
        nc.vector.tensor_sub(lg[:], lg[:], pprod[:])
        cc = sp.tile([BL, E], F32, tag="cc")
        nc.vector.tensor_sub(cc[:], lg[:, 2 * E:30], lg[:, 0:E])
        nc.vector.tensor_sub(cc[:], cc[:], lg[:, E:2 * E])
        am1 = sp.tile([BL, E], F32, tag="am1")
        bm1 = sp.tile([BL, E], F32, tag="bm1")
        nc.vector.tensor_scalar_add(am1[:], x3[:, 0:E], -1.0)
        nc.vector.tensor_scalar_add(bm1[:], x3[:, E:2 * E], -1.0)
        lpv = W[:, :, 0:T]
        lp2 = sp.tile([BL, E, T], F32, tag="lp2")
        nc.vector.tensor_mul(
            lpv, am1[:].unsqueeze(-1).to_broadcast([BL, E, T]),
            logt[:, 0:T].unsqueeze(1).to_broadcast([BL, E, T]))
        nc.vector.tensor_mul(
            lp2[:], bm1[:].unsqueeze(-1).to_broadcast([BL, E, T]),
            log1mt[:, 0:T].unsqueeze(1).to_broadcast([BL, E, T]))
        nc.vector.tensor_add(lpv, lpv, lp2[:])
        nc.vector.tensor_add(
            lpv, lpv, cc[:].unsqueeze(-1).to_broadcast([BL, E, T]))
        nc.scalar.activation(lpv, lpv, AF.Exp)
        wmax = sp.tile([BL, E], F32, tag="wmax")
        nc.vector.tensor_reduce(wmax[:], lpv, AX.X, ALU.max)
        nc.vector.tensor_scalar_add(wmax[:], wmax[:], 1e-8)
        winv = sp.tile([BL, E], F32, tag="winv")
        nc.vector.reciprocal(winv[:], wmax[:])
        nc.vector.tensor_mul(winv[:], winv[:], p_sel[:])
        nc.vector.tensor_mul(
            lpv, lpv, winv[:].unsqueeze(-1).to_broadcast([BL, E, T]))
        nc.vector.memset(W[:, :, T:TP], 0.0)
        W_t = sp.tile([BL, TP, E], F32, tag="lp2", name="W_t")
        nc.vector.tensor_copy(W_t[:], W[:].transpose([0, 2, 1]))
        dmaA(d["w_scr"].ap().rearrange("b (t e) -> b t e", e=E), W_t[:])
        # wp reload: [tok, jt, e], 2 DMAs
        wsv = d["w_scr"].ap().rearrange(
            "(jt two) (t e) -> two t jt e", two=2, e=E)
        for i in range(2):
            dmaA(wp[64 * i:64 * i + 64], wsv[i])
        # Wones: per-expert weighted block-ones, bf16
        wonev = wones[:]
        for jt in range(NT):
            eng = nc.vector if jt % 2 == 0 else nc.gpsimd
            eng.tensor_mul(
                wonev[:, jt, :].rearrange("p (e b) -> p e b", e=E),
                ones2.unsqueeze(1).to_broadcast([128, E, 2]),
                wp[:, jt, :].unsqueeze(-1).to_broadcast([128, E, 2]))
        hp2_mgr.__exit__(None, None, None)

        # preload the sqrt ACT table off the critical path (the tail
        # LayerNorm Sqrt would otherwise pay a 1.28us table switch)
        sqpre = sp.tile([1, 1], F32, tag="sqpre")
        nc.scalar.activation(sqpre[:], wp[0:1, 0, 0:1], AF.Sqrt)

        mm1_nch(2, ("act", "act", "vector"))
        mm1_nch(3, ("act", "act", "vector"))
        mm1_nch(4, ("act", "act", "vector"))

        # ================= pooling (gT accumulate) + mm2, interleaved
        w2p = tc.tile_pool(name="w2p", bufs=1)
        w2pool = w2p.__enter__()
        w2_f = w2pool.tile([128, 20 * C], BF16, tag="w2")
        dma(w2_f[:], d["d_w2"].ap())
        w2 = w2_f[:].rearrange("p (k n) -> p k n", k=20)
        if use_b2:
            b2c = w2pool.tile([E, C], F32R, tag="b2c")
            dma(b2c[:], d["d_b2"].ap())
        if use_ln:
            lngb = w2pool.tile([BL, 2 * C], F32, tag="lngb")
            dma(lngb[:], d["d_lng"].ap())

        with tc.tile_pool(name="psC", bufs=1, space="PSUM") as pC:
            poT = pC.tile([128, 4 * BL], F32, tag="poT")
            gpsA = pC.tile([128, 16 * BL], F32, tag="gpsA", name="gpsA")
            gpsB = pC.tile([128, 4 * BL], F32, tag="gpsB", name="gpsB")
            for ct in range(20):
                e = ct // 2
                gps = gpsA[:, ct * BL:(ct + 1) * BL] if ct < 16 else \
                    gpsB[:, (ct - 16) * BL:(ct - 15) * BL]
                for jt in range(NT):
                    nc.tensor.matmul(
                        gps[:, 2 * jt:2 * jt + 2],
                        swhv[:, jt, ct * 128:(ct + 1) * 128],
                        wonev[:, jt, 2 * e:2 * e + 2],
                        start=True, stop=True, skip_group_check=True)
            nc.scalar.copy(gT[:, 0:16, :], gpsA[:].rearrange(
                "p (c b) -> p c b", b=BL))
            nc.scalar.copy(gT[:, 16:20, :], gpsB[:].rearrange(
                "p (c b) -> p c b", b=BL))
            for cti in range(4):
                for kt in range(20):
                    nc.tensor.matmul(
                        poT[:, cti * BL:(cti + 1) * BL],
                        w2[:, kt, cti * 128:(cti + 1) * 128],
                        gT[:, kt, :],
                        start=(kt == 0), stop=(kt == 19 and not use_b2))
            if use_b2:
                sbe = sp.tile([BL, E], F32R, tag="sbe")
                with nc.allow_low_precision(reason="fp32r sbe"):
                    nc.vector.tensor_reduce(sbe[:], W[:].bitcast(F32R),
                                            AX.X, ALU.add)
                pst = pC.tile([128, BL], F32R, tag="gT2", bufs=1)
                nc.tensor.transpose(pst[0:E, :], sbe[:], eyef[0:32, 0:32])
                sbeT = sp.tile([E, BL], F32R, tag="sbeT")
                nc.scalar.copy(sbeT[:], pst[0:E, :])
                for cti in range(4):
                    nc.tensor.matmul(
                        poT[:, cti * BL:(cti + 1) * BL],
                        b2c[:, cti * 128:(cti + 1) * 128],
                        sbeT[:], start=False, stop=True)
            # transpose poT back to [b, c] for the LayerNorm
            poTs = sp.tile([128, 4 * BL], F32R, tag="poTs")
            nc.scalar.copy(poTs[:], poT[:].bitcast(F32R))
            po = pC.tile([BL, C], F32, tag="po")
            mups = pC.tile([BL, 2], F32, tag="gpsB", name="mups")
            for cti in range(4):
                nc.tensor.matmul(
                    po[:, cti * 128:(cti + 1) * 128].bitcast(F32R),
                    poTs[:, cti * BL:(cti + 1) * BL],
                    eyef, is_transpose=True, skip_group_check=True)
                nc.tensor.matmul(mups[:], poTs[:, cti * BL:(cti + 1) * BL],
                                 onep, start=(cti == 0), stop=(cti == 3))
            # layernorm: Square fused with -mu bias, final (po-mu)*rstd
            mun = sp.tile([BL, 1], F32, tag="mun")
            nc.vector.tensor_scalar_mul(mun[:], mups[:, 0:1], -1.0 / C)
            sq = sp.tile([BL, C], F32, tag="sq")
            var = sp.tile([BL, 1], F32, tag="var")
            nc.scalar.activation(sq[:], po[:], AF.Square, bias=mun[:],
                                 accum_out=var[:])
            nc.vector.tensor_scalar(var[:], var[:], 1.0 / C, 1e-5,
                                    ALU.mult, ALU.add)
            sdv = sp.tile([BL, 1], F32, tag="sdv")
            nc.scalar.activation(sdv[:], var[:], AF.Sqrt)
            rstd = sp.tile([BL, 1], F32, tag="rstd")
            nc.vector.reciprocal(rstd[:], sdv[:])
            if use_ln:
                nc.vector.tensor_scalar(out_sb[:], po[:], mun[:], rstd[:],
                                        ALU.add, ALU.mult)
                nc.vector.tensor_mul(out_sb[:], out_sb[:], lngb[:, 0:C])
                nc.vector.tensor_add(out_sb[:], out_sb[:],
                                     lngb[:, C:2 * C])
                dma(d["d_out"].ap(), out_sb[:])
            else:
                for hf in range(2):
                    csl = slice(hf * 256, (hf + 1) * 256)
                    nc.vector.tensor_scalar(out_sb[:, csl], po[:, csl],
                                            mun[:], rstd[:],
                                            ALU.add, ALU.mult)
                    dma(d["d_out"].ap()[:, csl], out_sb[:, csl])
        w2p.__exit__(None, None, None)


def _split8(x):
    f8 = ml_dtypes.float8_e4m3
    hi = np.asarray(x, np.float32).astype(f8)
    lo = (np.asarray(x, np.float32) - hi.astype(np.float32)).astype(f8)
    return hi, lo


def _host_prep(inputs):
    f32 = np.float32
    qst = np.asarray(inputs["qst"], f32)
    data = np.asarray(inputs["data"], f32)
    in_proj_w = np.asarray(inputs["in_proj_w"], f32)
    in_proj_b = np.asarray(inputs["in_proj_b"], f32)
    out_proj_w = np.asarray(inputs["out_proj_w"], f32)
    out_proj_b = np.asarray(inputs["out_proj_b"], f32)
    router_w = np.asarray(inputs["router_w"], f32)
    router_b = np.asarray(inputs["router_b"], f32)
    beta_w = np.asarray(inputs["beta_w"], f32)
    beta_b = np.asarray(inputs["beta_b"], f32)
    exp_w1 = np.asarray(inputs["exp_w1"], f32)
    exp_b1 = np.asarray(inputs["exp_b1"], f32)
    exp_w2 = np.asarray(inputs["exp_w2"], f32)
    exp_b2 = np.asarray(inputs["exp_b2"], f32)
    ln_g = np.asarray(inputs["ln_g"], f32)
    ln_b = np.asarray(inputs["ln_b"], f32)
    assert not np.any(exp_b1), "exp_b1 != 0 not supported by this kernel"

    s = 1.0 / math.sqrt(DH)
    wq, wk, wv = np.split(in_proj_w.astype(np.float64), 3, axis=0)
    bq, bk, bv = np.split(in_proj_b.astype(np.float64), 3)
    opw = out_proj_w.astype(np.float64)
    c0 = opw @ bv + out_proj_b
    Wro = router_w @ opw
    bro = router_w.astype(np.float64) @ c0 + router_b
    Wbo = beta_w @ opw
    bbo = beta_w.astype(np.float64) @ c0 + beta_b

    def pad_k(mat_rows513, ncol):  # (513, ncol) -> (128, 5*ncol)
        out = np.zeros((5, 128, ncol), f32)
        out[0:4] = mat_rows513[0:512].reshape(4, 128, ncol)
        out[4, 0] = mat_rows513[512]
        return out.transpose(1, 0, 2).reshape(128, 5 * ncol)

    def tile_k(mat512, ncol):  # (512, ncol) -> (128, 4*ncol)
        return np.ascontiguousarray(
            mat512.reshape(NKC, 128, ncol).transpose(1, 0, 2)).reshape(
            128, NKC * ncol)

    wh = np.vstack([np.hstack([Wro.T, Wbo.T]),
                    np.hstack([bro, bbo])[None, :]]).astype(f32)

    apack = np.zeros((128, APK), f32)
    apack[:, A_WV:A_WV + NKC * C] = tile_k(
        np.ascontiguousarray(wv.T.astype(f32)), C)
    apack[0:64, A_WH:A_WH + 8 * 30] = np.ascontiguousarray(
        wh[0:512].reshape(8, 64, 30).transpose(1, 0, 2)).reshape(64, 240)
    apack[0, A_HB:A_HB + 30] = wh[512]

    cpack = np.zeros((128, CPK), f32)
    cpack[:, C_EYE:C_EYE + 128] = np.eye(128, dtype=f32)
    t = np.linspace(0.0, 1.0, T).astype(f32)
    logt = np.zeros(TP, f32); logt[:T] = np.log(t + 1e-12)
    log1mt = np.zeros(TP, f32); log1mt[:T] = np.log(1.0 - t + 1e-12)
    cpack[0:BL, C_LT:C_LT + TP] = logt[None, :]
    cpack[0:BL, C_L1:C_L1 + TP] = log1mt[None, :]
    onesBig = np.zeros((128, 62), f32)
    for p in range(128):
        if (p % TP) < T:
            onesBig[p, 30 + p // TP] = 1.0
    cpack[:, C_OB:C_OB + 62] = onesBig
    padmask = (np.arange(128) % TP < T).astype(f32)
    cpack[:, C_OC:C_OC + 32] = padmask[:, None]
    cpack[0, C_OR:C_OR + 128] = 1.0
    cpack[0, C_O12:C_O12 + 32] = 1.0
    for p in range(128):
        if (p % TP) < T:
            cpack[p, C_IM + (p // TP) * 8:C_IM + (p // TP) * 8 + 8] = 1.0
    cpack[:, C_M16] = -16.0
    cpack[:, C_ONE:C_ONE + 2] = 1.0
    for p in range(128):
        if (p % TP) < T:
            cpack[p, C_O2 + p // TP] = 1.0

    # expert weights: w1 fp8 hi/lo, w2 bf16
    w1cat = np.ascontiguousarray(
        exp_w1.transpose(2, 0, 1).reshape(C, EC))
    w1h, w1l = _split8(w1cat)
    w18h = tile_k(w1h, EC)
    w18l = tile_k(w1l, EC)
    w2catT = np.ascontiguousarray(
        exp_w2.transpose(0, 2, 1).reshape(EC, C).astype(
            ml_dtypes.bfloat16).reshape(20, 128, C).transpose(1, 0, 2)).reshape(
        128, 20 * C)

    use_b2 = bool(np.any(exp_b2))
    use_ln = bool(np.any(ln_b) or np.any(ln_g != 1.0))

    shared = {"apack": apack, "cpack": cpack,
              "w18h": w18h, "w18l": w18l, "w2catT": w2catT}
    if use_b2:
        shared["b2cat"] = exp_b2.copy()
    if use_ln:
        shared["lng"] = np.concatenate(
            [np.broadcast_to(ln_g, (BL, C)), np.broadcast_to(ln_b, (BL, C))],
            axis=1).astype(f32)

    in_maps = []
    for ci in range(NCORE):
        qst_l = qst[ci * BL:(ci + 1) * BL].astype(np.float64)
        data_l = data[ci * BL:(ci + 1) * BL]
        pad = np.zeros((BL, TP, C), f32)
        pad[:, :T] = data_l
        flat = pad.reshape(NTOK, C)
        dataT = tile_k(np.ascontiguousarray(flat.T), NTOK)
        dT8h_, dT8l_ = _split8(dataT)
        datab = np.ascontiguousarray(
            flat.reshape(NT, 128, C).transpose(1, 0, 2)).reshape(128, NT * C)
        # scores projector qk[b,h,:] = q_scaled[b,h,:] @ wk_h
        q_scaled = (qst_l @ wq.T + bq) * s                      # (BL, C)
        qk = np.einsum("bhd,hdc->bhc",
                       q_scaled.reshape(BL, H, DH),
                       wk.reshape(H, DH, C))                    # (BL, H, C)
        # col = 64*(b//8) + 16*((b//2)%4) + 2h + b%2
        qkcols = np.zeros((C, BL * H), f32)
        for b in range(BL):
            for h in range(H):
                col = 64 * (b // 8) + 16 * ((b // 2) % 4) + 8 * (b % 2) + h
                qkcols[:, col] = qk[b, h, :]
        m = {"dataT": dataT, "datab": datab,
             "dT8h": dT8h_, "dT8l": dT8l_,
             "qkT": tile_k(qkcols, BL * H)}
        m.update(shared)
        in_maps.append(m)
    return in_maps, use_b2, use_ln


def kernel(**inputs):
    in_maps, use_b2, use_ln = _host_prep(inputs)
    key = (use_b2, use_ln)
    if key not in _CACHE:
        _CACHE[key] = _build_program(use_b2, use_ln)
    nc = _CACHE[key]
    res = run_bass_kernel_spmd(nc, in_maps, core_ids=list(range(NCORE)))
    out = np.concatenate(
        [r["out"].reshape(BL, 1, C) for r in res.results], axis=0)
    return out.astype(np.float32)
